# revision 7
# baseline (speedup 1.0000x reference)
"""Trainium2 Bass kernel for nn_KernelMachine (random-feature kernel machine).

Computes out = (sqrt(2/N) * cos(x @ Wf^T + bf)) @ Wp on 8 NeuronCores,
data-parallel over the batch dimension (1024 rows/core), no collectives.

Per-core pipeline, per 128-wide tile of the N=4096 feature dim, with the
elementwise work load-balanced across THREE engines (DVE / Pool+Act / Act):

  MM1 (TensorE, f32r): t = [x | 1] @ [Wf/2pi | bf']  -> PSUM [128,1024] (turns)
      (bias and the cos->sin quarter-turn fold into the ones-row weights)
  then one of three per-tile paths to g ~ sin(2pi t):
   - path 'd' (DVE): one fused custom op: r = t - rne(t) via the 1.5*2^23
     magic constant, then the factored quintic r*(A-r^2)*(B-r^2) which is
     sin(2pi r)/K for K=53.52...  (K folds into this tile's Wp slice)
   - path 'p' (Pool+PE+Act): Pool tensor_scalar u=(t+m)-m (=rne(t), bf16),
     PE accumulates t-u into the same PSUM via a -I matmul, Act applies Sin.
   - path 'a' (Act+PE): like 'p' but u computed by two Act Identity passes.
  MM2 (TensorE, bf16, transposed): out[b,m] accumulated as 8 tiny
      [128b x 8m] matmuls per tile (lhsT = g columns, rhs = Wp tile);
      cost scales with the 8-wide moving dim instead of the 1024 batch.
"""
import sys

if "/opt/trn_rl_repo" not in sys.path:
    sys.path.insert(0, "/opt/trn_rl_repo")

import ml_dtypes
import numpy as np

import concourse.bacc as bacc
import concourse.mybir as mybir
import concourse.tile as tile
from concourse import bass_utils
from concourse import dve_ops
from concourse.dve_spec import C0, C1, C2, Spec, Src0, lower
from concourse.dve_uop import DveOpSpec

# Problem shape (hardcoded per contest contract).
B = 8192
D = 64
DA = D + 1  # contraction augmented with a ones-row (bias)
N = 4096
M = 8
NCORES = 8
BS = B // NCORES  # 1024 batch rows per core
P = 128
NT = N // P  # 32 feature tiles
FREE = 512  # MM1 matmul moving free dim (one PSUM bank)
NBG = BS // P  # 8 batch groups for the transposed MM2

f32 = mybir.dt.float32
f32r = mybir.dt.float32r
bf16 = mybir.dt.bfloat16

MAGIC = float(1.5 * 2**23)  # fp32 round-to-nearest-int magic constant
# sin(pi z) ~= k z (A - z^2)(B - z^2) minimax fit on [-1,1]; substituting
# z = 2r gives sin(2pi r) ~= KP * r (AQ - r^2)(BQ - r^2) on r in [-.5,.5].
AQ = 0.2512187245830011
BQ = 0.4600290215280054
KP = 53.520624390078666
# One fp32 ulp below 2*pi so |r|<=0.5 keeps Sin's argument inside [-pi,pi].
SCALE_SIN = float(np.nextafter(np.float32(2 * np.pi), np.float32(0)))

# Per-tile path assignment: 'd' = DVE quintic, 'p' = Act+Pool+PE+Act, 'a' = Act+PE.
ND, NA, NP = 21, 0, 11


def _make_paths(nd=ND, na=NA, np_=NP):
    """Spread the three path classes evenly over the NT tiles."""
    assert nd + na + np_ == NT
    counts = {"d": nd, "a": na, "p": np_}
    emitted = {"d": 0, "a": 0, "p": 0}
    out = []
    for t in range(NT):
        # pick the class whose emitted fraction lags its target most
        best, bestdef = None, None
        for k in ("d", "p", "a"):
            if counts[k] == 0:
                continue
            deficit = counts[k] * (t + 1) / NT - emitted[k]
            if bestdef is None or deficit > bestdef:
                best, bestdef = k, deficit
        out.append(best)
        emitted[best] += 1
        counts[best] -= 1
    return out


def _make_sine_op():
    """Custom DVE op: r = t - rne(t) (C0 = 1.5*2^23), out = r(C1-r^2)(C2-r^2).

    Registered into concourse.dve_ops at import time (idempotent)."""
    name = "SINE_QUINTIC_ANT"
    for o in dve_ops.OPS:
        if o.name == name:
            return o
    t = Src0
    u0 = t + C0
    u1 = u0 - C0
    r = t - u1
    s = r * r
    body = (r * (C1 - s)) * (C2 - s)

    def ref(in0, in1, s0, s1, imm2):
        tt = in0.astype(np.float32)
        u1 = ((tt + np.float32(s0)) - np.float32(s0)).astype(np.float32)
        r = (tt - u1).astype(np.float32)
        s = (r * r).astype(np.float32)
        return (
            (r * (np.float32(s1) - s)).astype(np.float32) * (np.float32(imm2) - s)
        ).astype(np.float32)

    spec = Spec(body=body, reference=ref)
    opcode = dve_ops._CUSTOM_DVE_ROW_BASE + len(dve_ops.OPS)
    dve_ops._SUB_OPCODE_FOR_NAME[name] = opcode
    shas = {}
    for ver in ("v3", "v4"):
        tmp = DveOpSpec(
            name=name, opcode=opcode, uops=lower(spec, ver=ver), rd1_en=False
        )
        shas[ver] = tmp.sha(ver)
    op = dve_ops.DveOp(name, spec, subdim=False, uops_sha=shas)
    dve_ops.OPS.append(op)
    dve_ops.CUSTOM_DVE_SPECS[name] = spec
    return op


def build(paths=None):
    paths = paths or _make_paths()
    sine_op = _make_sine_op()
    Ident = mybir.ActivationFunctionType.Identity
    Sin = mybir.ActivationFunctionType.Sin
    nc = bacc.Bacc("TRN2", target_bir_lowering=False, debug=False, num_devices=NCORES)

    xt_d = nc.dram_tensor("xt", [DA, BS], f32, kind="ExternalInput").ap()
    wft_d = nc.dram_tensor("wft", [DA, N], f32, kind="ExternalInput").ap()
    wps_d = nc.dram_tensor("wps", [P, NT, M], bf16, kind="ExternalInput").ap()
    negi_d = nc.dram_tensor("negi", [P, P], bf16, kind="ExternalInput").ap()
    out_d = nc.dram_tensor("out", [P, NBG, M], f32, kind="ExternalOutput").ap()

    with tile.TileContext(nc) as tc:
        with (
            tc.tile_pool(name="singles", bufs=1) as singles,
            tc.tile_pool(name="gpool", bufs=4) as gpool,
            tc.tile_pool(name="upool", bufs=3) as upool,
            tc.tile_pool(name="u0pool", bufs=2) as u0pool,
            tc.tile_pool(name="fpsum", bufs=3, space="PSUM") as fpsum,
            tc.tile_pool(name="accpool", bufs=1, space="PSUM") as accpool,
        ):
            # Preload weights; chunked so DMA queues parallelize and the
            # first matmuls can start before all chunks land.
            xt_tiles = []
            for j in range(2):
                t_ = singles.tile([DA, FREE], f32r, tag=f"xt{j}")
                nc.sync.dma_start(t_, xt_d[:, j * FREE:(j + 1) * FREE].bitcast(f32r))
                xt_tiles.append(t_)
            negi_sb = singles.tile([P, P], bf16, tag="negi")
            nc.sync.dma_start(negi_sb, negi_d)
            magic_sb = singles.tile([P, 1], f32, tag="magicsb")
            nc.gpsimd.memset(magic_sb[:], MAGIC)
            nmagic_sb = singles.tile([P, 1], f32, tag="nmagicsb")
            nc.gpsimd.memset(nmagic_sb[:], -MAGIC)
            wps_sb = singles.tile([P, NT, M], bf16, tag="wps")
            nc.sync.dma_start(wps_sb, wps_d)
            wft_tiles = []
            for c in range(8):
                t_ = singles.tile([DA, FREE], f32r, tag=f"wft{c}")
                nc.sync.dma_start(t_, wft_d[:, c * FREE:(c + 1) * FREE].bitcast(f32r))
                wft_tiles.append(t_)

            acc = accpool.tile([P, NBG * M], f32)

            fps_by_t = {}
            g_by_t = {}
            u_by_t = {}

            def emit_mm1(t):
                fps = fpsum.tile([P, BS], f32)
                fps_by_t[t] = fps
                helper = paths[t] != "d"
                lhsT = wft_tiles[t // 4][:, (t % 4) * P:(t % 4 + 1) * P]
                for j in range(2):
                    nc.tensor.matmul(
                        fps[:, j * FREE:(j + 1) * FREE],
                        lhsT=lhsT,
                        rhs=xt_tiles[j][:],
                        start=True,
                        stop=not helper,
                    )

            def emit_stage1(t):
                # produce either g (path d) or u (paths p/a) from fps
                fps = fps_by_t[t]
                if paths[t] == "d":
                    g = gpool.tile([P, BS], bf16)
                    g_by_t[t] = g
                    nc.vector._custom_dve(
                        sine_op, out=g[:], in0=fps[:], s0=MAGIC, s1=AQ, imm2=BQ
                    )
                else:  # 'p' / 'a': u0 = t + magic on Act (PSUM -> SBUF, rounds)
                    u0 = u0pool.tile([P, BS], f32)
                    nc.scalar.activation(u0[:], fps[:], Ident, bias=magic_sb[:])
                    u = upool.tile([P, BS], bf16)
                    u_by_t[t] = u
                    if paths[t] == "p":
                        # u = u0 - magic on Pool (SBUF only; GPSIMD can't see PSUM)
                        nc.gpsimd.tensor_scalar(
                            out=u[:], in0=u0[:], scalar1=MAGIC, scalar2=None,
                            op0=mybir.AluOpType.subtract,
                        )
                    else:
                        nc.scalar.activation(u[:], u0[:], Ident, bias=nmagic_sb[:])

            def emit_stage2(t):
                # helper paths: PE subtract (t - u -> r in psum), then Act Sin
                if paths[t] == "d":
                    return
                fps = fps_by_t[t]
                u = u_by_t[t]
                for j in range(2):
                    nc.tensor.matmul(
                        fps[:, j * FREE:(j + 1) * FREE],
                        lhsT=negi_sb[:],
                        rhs=u[:, j * FREE:(j + 1) * FREE],
                        start=False,
                        stop=(j == 1),
                    )
                g = gpool.tile([P, BS], bf16)
                g_by_t[t] = g
                nc.scalar.activation(g[:], fps[:], Sin, scale=SCALE_SIN)

            def emit_mm2(t):
                g = g_by_t[t]
                for bg in range(NBG):
                    nc.tensor.matmul(
                        acc[:, bg * M:(bg + 1) * M],
                        lhsT=g[:, bg * P:(bg + 1) * P],
                        rhs=wps_sb[:, t],
                        start=(t == 0 and bg == 0),
                        stop=(t == NT - 1 and bg == NBG - 1),
                        skip_group_check=True,
                    )

            # Software-pipelined emission: stage2 lags 2 tiles, mm2 lags 3, so
            # the PE/Act streams never block on freshly-issued upstream work.
            for t in range(NT + 3):
                if t < NT:
                    emit_mm1(t)
                    emit_stage1(t)
                if 0 <= t - 2 < NT:
                    emit_stage2(t - 2)
                if 0 <= t - 3 < NT:
                    emit_mm2(t - 3)

            out_sb = singles.tile([P, NBG * M], f32, tag="outsb")
            nc.any.tensor_copy(out=out_sb[:], in_=acc[:])
            nc.sync.dma_start(out_d, out_sb[:])
    nc.compile()
    return nc


_NC = None


def _get_nc():
    global _NC
    if _NC is None:
        _NC = build()
    return _NC


def _prep_in_maps(x, Wf, bf, Wp, paths=None):
    paths = paths or _make_paths()
    scale = np.float64(np.sqrt(2.0 / N))
    inv2pi = np.float64(1.0) / (2.0 * np.pi)
    # [65, 4096]: rows 0-63 = (Wf/2pi)^T, row 64 = bf/2pi + 1/4 (cos->sin)
    wft = np.empty((DA, N), dtype=np.float32)
    wft[:D] = (Wf.astype(np.float64) * inv2pi).astype(np.float32).T
    wft[D] = (bf.astype(np.float64) * inv2pi + 0.25).astype(np.float32)
    # Wp scaled per tile: DVE-path tiles additionally absorb the quintic's
    # leading coefficient KP.  [128, NT, M] in bf16.
    wps64 = Wp.astype(np.float64).reshape(NT, P, M) * scale
    for t in range(NT):
        if paths[t] == "d":
            wps64[t] *= KP
    wps = np.ascontiguousarray(wps64.transpose(1, 0, 2)).astype(ml_dtypes.bfloat16)
    negi = (-np.eye(P, dtype=np.float32)).astype(ml_dtypes.bfloat16)
    in_maps = []
    for c in range(NCORES):
        xt = np.empty((DA, BS), dtype=np.float32)
        xt[:D] = x[c * BS:(c + 1) * BS].astype(np.float32).T
        xt[D] = 1.0
        in_maps.append({"xt": xt, "wft": wft, "wps": wps, "negi": negi})
    return in_maps


def run(x, Wf, bf, Wp, trace=False):
    nc = _get_nc()
    in_maps = _prep_in_maps(x, Wf, bf, Wp)
    res = bass_utils.run_bass_kernel_spmd(
        nc, in_maps, core_ids=list(range(NCORES)), trace=trace
    )
    out = np.empty((B, M), dtype=np.float32)
    for c in range(NCORES):
        # device out: [P, NBG, M] with out[bg*128+p, m] = dev[p, bg, m]
        dev = res.results[c]["out"]
        out[c * BS:(c + 1) * BS, :] = dev.transpose(1, 0, 2).reshape(BS, M)
    return out, res


def kernel(x, Wf, bf, Wp):
    x = np.asarray(x)
    Wf = np.asarray(Wf)
    bf = np.asarray(bf)
    Wp = np.asarray(Wp)
    out, _ = run(x, Wf, bf, Wp, trace=False)
    return out


# revision 9
# speedup vs baseline: 1.2045x; 1.2045x over previous
"""Trainium2 Bass kernel for nn_KernelMachine (random-feature kernel machine).

Computes out = (sqrt(2/N) * cos(x @ Wf^T + bf)) @ Wp on 8 NeuronCores,
data-parallel over the batch dimension (1024 rows/core), no collectives.

Per-core pipeline, per 128-wide tile of the N=4096 feature dim, with the
elementwise work load-balanced across DVE and the Activation engine:

  MM1 (TensorE, f32r): t = [x | 1] @ [Wf/2pi | bf']  -> PSUM (in turns)
      (bias and the cos->sin quarter-turn fold into the ones-row weights)
  then one of two per-tile paths to g ~ sin(2pi t):
   - path 'd' (DVE): one fused custom op on [128,1024]: r = t - rne(t) via
     the 1.5*2^23 magic constant, then the factored quintic r(A-r^2)(B-r^2)
     which is sin(2pi r)/K for K=53.52...  (K folds into this tile's Wp)
   - path 'h' (Act+Pool+PE, two [128,512] halves): Act u0 = t + 1.5*2^23
     (= magic + rne(t), SBUF), Pool u = u0 - magic (bf16, small ints),
     PE accumulates t - u into the same PSUM bank via a -I matmul, Act
     applies Sin.
  MM2 (TensorE, bf16, transposed): out[b,m] accumulated as 8 tiny
      [128b x 8m] matmuls per tile (lhsT = g columns, rhs = Wp tile);
      cost scales with the 8-wide moving dim instead of the 1024 batch.

PSUM: d-tiles 2 bufs x 2 banks, helper halves 3 bufs x 1 bank, acc 1 bank.
"""
import sys

if "/opt/trn_rl_repo" not in sys.path:
    sys.path.insert(0, "/opt/trn_rl_repo")

import ml_dtypes
import numpy as np

import concourse.bacc as bacc
import concourse.mybir as mybir
import concourse.tile as tile
from concourse import bass_utils
from concourse import dve_ops
from concourse.dve_spec import C0, C1, C2, Spec, Src0, lower
from concourse.dve_uop import DveOpSpec

# Problem shape (hardcoded per contest contract).
B = 8192
D = 64
DA = D + 1  # contraction augmented with a ones-row (bias)
N = 4096
M = 8
NCORES = 8
BS = B // NCORES  # 1024 batch rows per core
P = 128
NT = N // P  # 32 feature tiles
FREE = 512  # MM1 matmul moving free dim (one PSUM bank)
NBG = BS // P  # 8 batch groups for the transposed MM2

f32 = mybir.dt.float32
f32r = mybir.dt.float32r
bf16 = mybir.dt.bfloat16
i16 = mybir.dt.int16  # unused by matmul (invalid dtype there)

MAGIC = float(1.5 * 2**23)  # fp32 round-to-nearest-int magic constant
# sin(pi z) ~= k z (A - z^2)(B - z^2) minimax fit on [-1,1]; substituting
# z = 2r gives sin(2pi r) ~= KP * r (AQ - r^2)(BQ - r^2) on r in [-.5,.5].
AQ = 0.2512187245830011
BQ = 0.4600290215280054
KP = 53.520624390078666
# One fp32 ulp below 2*pi so |r|<=0.5 keeps Sin's argument inside [-pi,pi].
SCALE_SIN = float(np.nextafter(np.float32(2 * np.pi), np.float32(0)))

ND, NH = 21, 11  # DVE-path tiles vs Act+PE helper tiles


def _make_paths(nd=ND, nh=NH):
    """Spread the two path classes evenly over the NT tiles."""
    assert nd + nh == NT
    counts = {"d": nd, "h": nh}
    emitted = {"d": 0, "h": 0}
    out = []
    for t in range(NT):
        best, bestdef = None, None
        for k in ("d", "h"):
            if counts[k] == 0:
                continue
            deficit = (counts[k] + emitted[k]) * (t + 1) / NT - emitted[k]
            if bestdef is None or deficit > bestdef:
                best, bestdef = k, deficit
        out.append(best)
        emitted[best] += 1
        counts[best] -= 1
    return out


def _make_sine_op():
    """Custom DVE op: r = t - rne(t) (C0 = 1.5*2^23), out = r(C1-r^2)(C2-r^2).

    Registered into concourse.dve_ops at import time (idempotent)."""
    name = "SINE_QUINTIC_ANT"
    for o in dve_ops.OPS:
        if o.name == name:
            return o
    t = Src0
    u0 = t + C0
    u1 = u0 - C0
    r = t - u1
    s = r * r
    body = (r * (C1 - s)) * (C2 - s)

    def ref(in0, in1, s0, s1, imm2):
        tt = in0.astype(np.float32)
        u1 = ((tt + np.float32(s0)) - np.float32(s0)).astype(np.float32)
        r = (tt - u1).astype(np.float32)
        s = (r * r).astype(np.float32)
        return (
            (r * (np.float32(s1) - s)).astype(np.float32) * (np.float32(imm2) - s)
        ).astype(np.float32)

    spec = Spec(body=body, reference=ref)
    opcode = dve_ops._CUSTOM_DVE_ROW_BASE + len(dve_ops.OPS)
    dve_ops._SUB_OPCODE_FOR_NAME[name] = opcode
    shas = {}
    for ver in ("v3", "v4"):
        tmp = DveOpSpec(
            name=name, opcode=opcode, uops=lower(spec, ver=ver), rd1_en=False
        )
        shas[ver] = tmp.sha(ver)
    op = dve_ops.DveOp(name, spec, subdim=False, uops_sha=shas)
    dve_ops.OPS.append(op)
    dve_ops.CUSTOM_DVE_SPECS[name] = spec
    return op


def build(paths=None):
    paths = paths or _make_paths()
    sine_op = _make_sine_op()
    Ident = mybir.ActivationFunctionType.Identity
    Sin = mybir.ActivationFunctionType.Sin
    nc = bacc.Bacc("TRN2", target_bir_lowering=False, debug=False, num_devices=NCORES)

    xt_d = nc.dram_tensor("xt", [DA, BS], f32, kind="ExternalInput").ap()
    wft_d = nc.dram_tensor("wft", [DA, N], f32, kind="ExternalInput").ap()
    wps_d = nc.dram_tensor("wps", [P, NT, M], bf16, kind="ExternalInput").ap()
    negi_d = nc.dram_tensor("negi", [P, P], bf16, kind="ExternalInput").ap()
    out_d = nc.dram_tensor("out", [P, NBG, M], f32, kind="ExternalOutput").ap()

    with tile.TileContext(nc) as tc:
        with (
            tc.tile_pool(name="singles", bufs=1) as singles,
            tc.tile_pool(name="gpool", bufs=4) as gpool,
            tc.tile_pool(name="u0pool", bufs=4) as u0pool,
            tc.tile_pool(name="upool", bufs=4) as upool,
            tc.tile_pool(name="fpd", bufs=2, space="PSUM") as fpd,
            tc.tile_pool(name="fph", bufs=3, space="PSUM") as fph,
            tc.tile_pool(name="accpool", bufs=1, space="PSUM") as accpool,
        ):
            # Preload weights; chunked so DMA queues parallelize and the
            # first matmuls can start before all chunks land.
            xt_tiles = []
            for j in range(2):
                t_ = singles.tile([DA, FREE], f32r, tag=f"xt{j}")
                nc.sync.dma_start(t_, xt_d[:, j * FREE:(j + 1) * FREE].bitcast(f32r))
                xt_tiles.append(t_)
            negi_sb = singles.tile([P, P], bf16, tag="negi")
            nc.sync.dma_start(negi_sb, negi_d)
            wps_sb = singles.tile([P, NT, M], bf16, tag="wps")
            nc.sync.dma_start(wps_sb, wps_d)
            magic_sb = singles.tile([P, 1], f32, tag="magicsb")
            nc.gpsimd.memset(magic_sb[:], MAGIC)
            wft_tiles = []
            for c in range(8):
                t_ = singles.tile([DA, FREE], f32r, tag=f"wft{c}")
                nc.sync.dma_start(t_, wft_d[:, c * FREE:(c + 1) * FREE].bitcast(f32r))
                wft_tiles.append(t_)

            acc = accpool.tile([P, NBG * M], f32)

            fps_by_t = {}
            g_by_t = {}
            u0_by_t = {}

            def emit_mm1(t):
                lhsT = wft_tiles[t // 4][:, (t % 4) * P:(t % 4 + 1) * P]
                if paths[t] == "d":
                    fps = fpd.tile([P, BS], f32)
                    fps_by_t[t] = fps
                    for j in range(2):
                        nc.tensor.matmul(
                            fps[:, j * FREE:(j + 1) * FREE],
                            lhsT=lhsT,
                            rhs=xt_tiles[j][:],
                            start=True,
                            stop=True,
                        )
                else:
                    halves = []
                    for j in range(2):
                        fh = fph.tile([P, FREE], f32)
                        halves.append(fh)
                        nc.tensor.matmul(
                            fh[:],
                            lhsT=lhsT,
                            rhs=xt_tiles[j][:],
                            start=True,
                            stop=False,
                        )
                    fps_by_t[t] = halves

            def emit_stage1(t):
                # produce either g (path d) or u0 halves (path h) from psum
                if paths[t] == "d":
                    g = gpool.tile([P, BS], bf16)
                    g_by_t[t] = g
                    nc.vector._custom_dve(
                        sine_op, out=g[:], in0=fps_by_t[t][:], s0=MAGIC, s1=AQ, imm2=BQ
                    )
                else:
                    us = []
                    for j in range(2):
                        u0 = u0pool.tile([P, FREE], f32)
                        nc.scalar.activation(
                            u0[:], fps_by_t[t][j][:], Ident, bias=magic_sb[:]
                        )
                        u = upool.tile([P, FREE], bf16)
                        us.append(u)
                        nc.gpsimd.tensor_scalar(
                            out=u[:], in0=u0[:], scalar1=MAGIC, scalar2=None,
                            op0=mybir.AluOpType.subtract,
                        )
                    u0_by_t[t] = us

            def emit_stage2(t):
                # path h: PE subtract (t - rne(t) -> r in psum), then Act Sin
                if paths[t] == "d":
                    return
                g = gpool.tile([P, BS], bf16)
                g_by_t[t] = g
                for j in range(2):
                    nc.tensor.matmul(
                        fps_by_t[t][j][:],
                        lhsT=negi_sb[:],
                        rhs=u0_by_t[t][j][:],
                        start=False,
                        stop=True,
                    )
                for j in range(2):
                    nc.scalar.activation(
                        g[:, j * FREE:(j + 1) * FREE],
                        fps_by_t[t][j][:],
                        Sin,
                        scale=SCALE_SIN,
                    )

            def emit_mm2(t):
                g = g_by_t[t]
                for bg in range(NBG):
                    nc.tensor.matmul(
                        acc[:, bg * M:(bg + 1) * M],
                        lhsT=g[:, bg * P:(bg + 1) * P],
                        rhs=wps_sb[:, t],
                        start=(t == 0 and bg == 0),
                        stop=(t == NT - 1 and bg == NBG - 1),
                        skip_group_check=True,
                    )

            # Software-pipelined emission: stage2 lags 1 tile, mm2 lags 2.
            for t in range(NT + 2):
                if t < NT:
                    emit_mm1(t)
                    emit_stage1(t)
                if 0 <= t - 1 < NT:
                    emit_stage2(t - 1)
                if 0 <= t - 2 < NT:
                    emit_mm2(t - 2)

            out_sb = singles.tile([P, NBG * M], f32, tag="outsb")
            nc.any.tensor_copy(out=out_sb[:], in_=acc[:])
            nc.sync.dma_start(out_d, out_sb[:])
    nc.compile()
    return nc


_NC = None


def _get_nc():
    global _NC
    if _NC is None:
        _NC = build()
    return _NC


def _prep_in_maps(x, Wf, bf, Wp, paths=None):
    paths = paths or _make_paths()
    scale = np.float64(np.sqrt(2.0 / N))
    inv2pi = np.float64(1.0) / (2.0 * np.pi)
    # [65, 4096]: rows 0-63 = (Wf/2pi)^T, row 64 = bf/2pi + 1/4 (cos->sin)
    wft = np.empty((DA, N), dtype=np.float32)
    wft[:D] = (Wf.astype(np.float64) * inv2pi).astype(np.float32).T
    wft[D] = (bf.astype(np.float64) * inv2pi + 0.25).astype(np.float32)
    # Wp scaled per tile: DVE-path tiles additionally absorb the quintic's
    # leading coefficient KP.  [128, NT, M] in bf16.
    wps64 = Wp.astype(np.float64).reshape(NT, P, M) * scale
    for t in range(NT):
        if paths[t] == "d":
            wps64[t] *= KP
    wps = np.ascontiguousarray(wps64.transpose(1, 0, 2)).astype(ml_dtypes.bfloat16)
    negi = (-np.eye(P)).astype(ml_dtypes.bfloat16)
    in_maps = []
    for c in range(NCORES):
        xt = np.empty((DA, BS), dtype=np.float32)
        xt[:D] = x[c * BS:(c + 1) * BS].astype(np.float32).T
        xt[D] = 1.0
        in_maps.append({"xt": xt, "wft": wft, "wps": wps, "negi": negi})
    return in_maps


def run(x, Wf, bf, Wp, trace=False):
    nc = _get_nc()
    in_maps = _prep_in_maps(x, Wf, bf, Wp)
    res = bass_utils.run_bass_kernel_spmd(
        nc, in_maps, core_ids=list(range(NCORES)), trace=trace
    )
    out = np.empty((B, M), dtype=np.float32)
    for c in range(NCORES):
        # device out: [P, NBG, M] with out[bg*128+p, m] = dev[p, bg, m]
        dev = res.results[c]["out"]
        out[c * BS:(c + 1) * BS, :] = dev.transpose(1, 0, 2).reshape(BS, M)
    return out, res


def kernel(x, Wf, bf, Wp):
    x = np.asarray(x)
    Wf = np.asarray(Wf)
    bf = np.asarray(bf)
    Wp = np.asarray(Wp)
    out, _ = run(x, Wf, bf, Wp, trace=False)
    return out
